# revision 10
# baseline (speedup 1.0000x reference)
"""Trainium2 Bass kernel for nn_DividedModel (64 independent MLP towers).

Math (per tower o of O=64):
    h0 = relu(x @ W0[o] + b0[o])         x: [B, 32], W0[o]: [32, 64]
    h1 = relu(h0 @ W1[o] + b1[o])        W1[o]: [64, 64]
    h2 = relu(h1 @ W2[o] + b2[o])        W2[o]: [64, 64]
    out[:, o] = h2 @ W3[o] + b3[o]       W3[o]: [64]

Strategy:
  - Data-parallel: batch B=16384 sharded 8 ways (2048 rows/core), params
    replicated.
  - Activations are kept transposed ([h, batch]) so weights are the
    stationary matmul operand and no transposes are ever needed; the
    input x is transposed on the host, the output is produced as
    [O, B_local] and transposed back on the host.
  - Tower PAIRS are packed block-diagonally into single 128-wide matmuls:
    lhsT [128, 128] = diag(W[a], W[b]), rhs [128, 512] = [h_a; h_b].
  - b0 is folded into the L0 matmul via an extra ones-row of the input
    (contraction K=66: 32 x-rows + 1 ones-row per tower of the pair).
  - L3 is one accumulating matmul chain per batch chunk: 32 block
    matmuls (one per pair) summing into a single [64, 512] PSUM bank.
  - PSUM->SBUF relu evacuation (the throughput bottleneck) alternates
    between ScalarE ACTIVATE and VectorE tensor_scalar ops.
"""

import numpy as np

B, I, O, H = 16384, 32, 64, 64
NCORES = 8
BL = B // NCORES      # 2048 batch rows per core
NB = 512              # batch columns per matmul (one PSUM bank fp32)
CHUNKS = BL // NB     # 4
NPAIR = O // 2        # 32 tower pairs
NGROUP = O // 4       # 16 groups of 2 pairs
SKEW = 2              # software-pipeline slot skew between layers

_CACHE = {}


def _build(reps: int = 1, use_f32r: bool = True):
    import concourse.mybir as mybir
    import concourse.tile as tile
    from concourse import bacc

    f32 = mybir.dt.float32
    f32r = mybir.dt.float32r if use_f32r else f32
    Relu = mybir.ActivationFunctionType.Relu
    Ident = mybir.ActivationFunctionType.Identity
    add_op = mybir.AluOpType.add
    max_op = mybir.AluOpType.max

    nc = bacc.Bacc(None, target_bir_lowering=False, debug=False)

    x2_d = nc.dram_tensor("x2", [66, BL], f32r, kind="ExternalInput")
    wl0_d = nc.dram_tensor("wl0", [66, 128 * NPAIR], f32r, kind="ExternalInput")
    wl1_d = nc.dram_tensor("wl1", [128, 128 * NPAIR], f32r, kind="ExternalInput")
    wl2_d = nc.dram_tensor("wl2", [128, 128 * NPAIR], f32r, kind="ExternalInput")
    wl3_d = nc.dram_tensor("wl3", [128, 64 * NPAIR], f32r, kind="ExternalInput")
    bb_d = nc.dram_tensor("bb", [128, 65], f32, kind="ExternalInput")
    outT_d = nc.dram_tensor("outT", [O, BL], f32, kind="ExternalOutput")

    with tile.TileContext(nc) as tc:
        with (
            tc.tile_pool(name="w", bufs=1) as wpool,
            tc.tile_pool(name="h", bufs=16) as hpool,
            tc.tile_pool(name="ot", bufs=3) as opool,
            tc.tile_pool(name="pp", bufs=6, space="PSUM") as ppool,
            tc.tile_pool(name="l3", bufs=2, space="PSUM") as l3pool,
        ):
            x2_s = wpool.tile([128, BL], f32r, tag="x2")
            wl0_s = wpool.tile([128, 128 * NPAIR], f32r, tag="wl0")
            wl1_s = wpool.tile([128, 128 * NPAIR], f32r, tag="wl1")
            wl2_s = wpool.tile([128, 128 * NPAIR], f32r, tag="wl2")
            wl3_s = wpool.tile([128, 64 * NPAIR], f32r, tag="wl3")
            bb_s = wpool.tile([128, 65], f32, tag="bb")

            # Spread input loads across the three DMA-capable queues
            # (SP/sync, GpSimd, ScalarE); first-needed data goes first.
            half0 = 64 * NPAIR
            nc.sync.dma_start(bb_s[:], bb_d[:])
            nc.sync.dma_start(x2_s[0:66, :NB], x2_d[:, :NB])
            nc.gpsimd.dma_start(wl0_s[0:66, :half0], wl0_d[:, :half0])
            nc.scalar.dma_start(wl0_s[0:66, half0:], wl0_d[:, half0:])
            nc.scalar.dma_start(wl1_s[:, :half0], wl1_d[:, :half0])
            nc.gpsimd.dma_start(wl1_s[:, half0:], wl1_d[:, half0:])
            nc.sync.dma_start(x2_s[0:66, NB:], x2_d[:, NB:])
            nc.gpsimd.dma_start(wl2_s[:, :half0], wl2_d[:, :half0])
            nc.scalar.dma_start(wl2_s[:, half0:], wl2_d[:, half0:])
            nc.sync.dma_start(wl3_s[:], wl3_d[:])

            # Alternate evacuation between ScalarE (ACT) and VectorE (DVE);
            # ACT is slightly faster per op so it gets 9 of every 16.
            act_pat = [1, 0, 1, 0, 1, 0, 1, 0, 1, 0, 1, 0, 1, 0, 1, 1]
            evac_state = [0]

            def evac_relu(dst, src, bias_col):
                """dst[SBUF] = relu(src[PSUM] + bias), one [128, 512] bank."""
                evac_state[0] += 1
                if bias_col is None:
                    nc.any.tensor_scalar_max(dst, src, 0.0)
                else:
                    bias_ap = bb_s[:, bias_col : bias_col + 1]
                    nc.any.tensor_scalar(dst, src, bias_ap, 0.0, add_op, max_op)

            NLANES = CHUNKS * NPAIR  # 128 global lanes: (chunk, pair)
            for _rep in range(reps):
                l3ps = [None] * CHUNKS
                h0 = [None] * NLANES
                h1 = [None] * NLANES
                h2 = [None] * NLANES
                for s in range(NLANES + 3 * SKEW):
                    ln = s
                    if 0 <= ln < NLANES:
                        c, t = divmod(ln, NPAIR)
                        cs = slice(NB * c, NB * (c + 1))
                        if t == 0:
                            l3ps[c] = l3pool.tile([128, NB], f32, tag="l3",
                                                  name="l3p")
                        pp = ppool.tile([128, NB], f32, tag="pp", name="pp0")
                        nc.tensor.matmul(
                            pp[:],
                            wl0_s[0:66, 128 * t : 128 * (t + 1)],
                            x2_s[0:66, cs],
                            start=True,
                            stop=True,
                        )
                        h0[ln] = hpool.tile([128, NB], f32r, tag="h", name="h0")
                        evac_relu(h0[ln][:], pp[:], None)
                    ln = s - SKEW
                    if 0 <= ln < NLANES:
                        c, t = divmod(ln, NPAIR)
                        pp = ppool.tile([128, NB], f32, tag="pp", name="pp1")
                        nc.tensor.matmul(
                            pp[:],
                            wl1_s[:, 128 * t : 128 * (t + 1)],
                            h0[ln][:],
                            start=True,
                            stop=True,
                        )
                        h0[ln] = None
                        h1[ln] = hpool.tile([128, NB], f32r, tag="h", name="h1")
                        evac_relu(h1[ln][:], pp[:], 1 + t)
                    ln = s - 2 * SKEW
                    if 0 <= ln < NLANES:
                        c, t = divmod(ln, NPAIR)
                        pp = ppool.tile([128, NB], f32, tag="pp", name="pp2")
                        nc.tensor.matmul(
                            pp[:],
                            wl2_s[:, 128 * t : 128 * (t + 1)],
                            h1[ln][:],
                            start=True,
                            stop=True,
                        )
                        h1[ln] = None
                        h2[ln] = hpool.tile([128, NB], f32r, tag="h", name="h2")
                        evac_relu(h2[ln][:], pp[:], 33 + t)
                    ln = s - 3 * SKEW
                    if 0 <= ln < NLANES:
                        c, t = divmod(ln, NPAIR)
                        cs = slice(NB * c, NB * (c + 1))
                        nc.tensor.matmul(
                            l3ps[c][0:64, :],
                            wl3_s[:, 64 * t : 64 * (t + 1)],
                            h2[ln][:],
                            start=(t == 0),
                            stop=(t == NPAIR - 1),
                        )
                        h2[ln] = None
                        if t == NPAIR - 1:
                            out_sb = opool.tile([64, NB], f32, tag="ot")
                            nc.scalar.activation(
                                out_sb[:], l3ps[c][0:64, :], Ident,
                                bias=bb_s[0:64, 0:1],
                            )
                            l3ps[c] = None
                            nc.sync.dma_start(outT_d[:, cs], out_sb[:])

    nc.compile()
    return nc


def _prep_weights(W0, b0, W1, b1, W2, b2, W3, b3):
    WL0 = np.zeros((66, 128 * NPAIR), np.float32)
    WL1 = np.zeros((128, 128 * NPAIR), np.float32)
    WL2 = np.zeros((128, 128 * NPAIR), np.float32)
    WL3 = np.zeros((128, 64 * NPAIR), np.float32)
    bb = np.zeros((128, 65), np.float32)
    bb[0:64, 0] = b3
    for t in range(NPAIR):
        a, b = 2 * t, 2 * t + 1
        c0 = 128 * t
        WL0[0:32, c0 : c0 + 64] = W0[a]
        WL0[32, c0 : c0 + 64] = b0[a]
        WL0[33:65, c0 + 64 : c0 + 128] = W0[b]
        WL0[65, c0 + 64 : c0 + 128] = b0[b]
        WL1[0:64, c0 : c0 + 64] = W1[a]
        WL1[64:128, c0 + 64 : c0 + 128] = W1[b]
        WL2[0:64, c0 : c0 + 64] = W2[a]
        WL2[64:128, c0 + 64 : c0 + 128] = W2[b]
        WL3[0:64, 64 * t + a] = W3[a]
        WL3[64:128, 64 * t + b] = W3[b]
        bb[0:64, 1 + t] = b1[a]
        bb[64:128, 1 + t] = b1[b]
        bb[0:64, 33 + t] = b2[a]
        bb[64:128, 33 + t] = b2[b]
    return WL0, WL1, WL2, WL3, bb


def _prep_x(x):
    """Per-core [128, BL] tiles: x^T twice (rows 0:32 / 33:65) + ones rows."""
    xT = np.ascontiguousarray(np.asarray(x, np.float32).T)  # [I, B]
    tiles = []
    for core in range(NCORES):
        sl = xT[:, core * BL : (core + 1) * BL]
        t = np.zeros((66, BL), np.float32)
        t[0:32] = sl
        t[32] = 1.0
        t[33:65] = sl
        t[65] = 1.0
        tiles.append(t)
    return tiles


def kernel(x, W0, b0, W1, b1, W2, b2, W3, b3):
    from concourse.bass_utils import run_bass_kernel_spmd

    x, W0, b0, W1, b1, W2, b2, W3, b3 = (
        np.asarray(a, np.float32) for a in (x, W0, b0, W1, b1, W2, b2, W3, b3)
    )
    key = "nc"
    if key not in _CACHE:
        _CACHE[key] = _build()
    nc = _CACHE[key]

    WL0, WL1, WL2, WL3, bb = _prep_weights(W0, b0, W1, b1, W2, b2, W3, b3)
    xts = _prep_x(x)
    in_maps = [
        {"x2": xts[core], "wl0": WL0, "wl1": WL1, "wl2": WL2, "wl3": WL3, "bb": bb}
        for core in range(NCORES)
    ]
    res = run_bass_kernel_spmd(nc, in_maps, core_ids=list(range(NCORES)))
    out = np.concatenate(
        [r["outT"].T for r in res.results], axis=0
    )
    return np.ascontiguousarray(out, np.float32)


if __name__ == "__main__":
    rng = np.random.default_rng(0)
    inputs = {
        "x": rng.standard_normal((B, I), np.float32),
        "W0": rng.standard_normal((O, I, H), np.float32) / np.sqrt(I),
        "b0": np.zeros((O, H), np.float32),
        "W1": rng.standard_normal((O, H, H), np.float32) / np.sqrt(H),
        "b1": np.zeros((O, H), np.float32),
        "W2": rng.standard_normal((O, H, H), np.float32) / np.sqrt(H),
        "b2": np.zeros((O, H), np.float32),
        "W3": rng.standard_normal((O, H), np.float32) / np.sqrt(H),
        "b3": np.zeros((O,), np.float32),
    }
    out = kernel(**inputs)
    print(out.shape, out.dtype, float(np.abs(out).mean()))
